# revision 12
# baseline (speedup 1.0000x reference)
"""Trainium2 Bass kernel for nn_Attention_74766790689469.

Computation (per batch b):
    z   = LayerNorm(x) (no gamma/beta: gamma folded into W, beta==0 fast path)
    f_t = z_t @ W'^T               (t in {q,k,v}, W' = w_in * gamma)
    qn  = fq / |fq|_rows ; kn = fk / |fk|_rows    (per head, d=64)
    out_h = qn_h @ (kn_h^T @ fv_h)                (no softmax -> associativity)
    y   = concat_h(out_h) @ w_out^T + b_out

Sharding: 8 cores = 4 batches x 2 head-groups (4 heads each). Each core
computes a partial y for its (b, g); host sums the two group partials.

Key device-side structure (per core), all matmul operands fp16, PSUM f32:
  - load q,k,v [2048,512] f32 -> fp16 (SWDGE cast DMA)
  - LN stats via bn_stats/bn_aggr, affine in-place
  - PE-transpose z_q, z_k -> feature-major [512, 2048]
  - fk = z_k^T-proj (n-major), row norms, kn = fk * rk
  - G^T = sum_m z_v[m,:]^T kn[m,:]  ([512, 256], contracts over m -> no
    z_v transpose needed);  S_h = G_h @ W'_vh^T  [64,64] per head
  - fq^T = W' @ z_q^T (feature-major); q-norms via indicator matmul
  - U^T = blockdiag(S)^T-matmul with fq^T;  O^T = U^T * R (R = rq bcast)
  - y = O^T-proj with w_out^T, output fp16
"""

import os
import numpy as np

HEADS = 8
DIM_HEAD = 64
DIM = 512
N = 2048
B = 4
LN_EPS = 1e-5
GH = 4            # heads per group
GO = GH * DIM_HEAD  # 256 outputs per group
NT = N // 128     # 16 row tiles
IC = DIM // 128   # 4 input-feature chunks
OC = GO // 128    # 2 output-feature chunks per group

_CACHE = {}


def _build_program(debug=False):
    """Build the single-core Bass program (same for every core)."""
    import concourse.bass as bass
    import concourse.bacc as bacc
    import concourse.tile as tile
    from concourse import mybir

    fp16 = mybir.dt.float16
    f32 = mybir.dt.float32
    AF = mybir.ActivationFunctionType
    OP = mybir.AluOpType
    AX = mybir.AxisListType

    nc = bacc.Bacc(
        "TRN2",
        target_bir_lowering=False,
        debug=False,
        enable_asserts=False,
        num_devices=8,
    )

    # ---- DRAM I/O ----
    xq_d = nc.dram_tensor("xq", [N, DIM], f32, kind="ExternalInput").ap()
    xk_d = nc.dram_tensor("xk", [N, DIM], f32, kind="ExternalInput").ap()
    xv_d = nc.dram_tensor("xv", [N, DIM], f32, kind="ExternalInput").ap()
    wT_d = nc.dram_tensor("wT", [DIM, GO], fp16, kind="ExternalInput").ap()
    woT_d = nc.dram_tensor("woT", [GO, DIM], fp16, kind="ExternalInput").ap()
    e2_d = nc.dram_tensor("e2", [128, OC, 128], fp16, kind="ExternalInput").ap()
    id_d = nc.dram_tensor("ident", [128, 128], fp16, kind="ExternalInput").ap()
    y_d = nc.dram_tensor("y", [N, DIM], fp16, kind="ExternalOutput").ap()
    rqd = nc.dram_tensor("rqd", [128, N], fp16).ap()  # internal scratch
    dbg = {}
    if debug:
        for nm, shp in [
            ("dzk", [128, NT, DIM]), ("dzq", [128, NT, DIM]),
            ("dzv", [128, NT, DIM]), ("dzkT", [128, IC, N]),
            ("dzqT", [128, IC, N]), ("dfqT", [128, OC, N]),
            ("dkn", [128, NT, GO]), ("dgt", [128, IC, GO]),
            ("dsblk", [128, OC, 128]), ("drq", [128, N]),
            ("drr", [128, OC, N]), ("dot", [128, OC, N]),
        ]:
            dbg[nm] = nc.dram_tensor(nm, shp, fp16, kind="ExternalOutput").ap()

    # tiled views  (row n = 128*t + p)
    xq_r = xq_d.rearrange("(t p) c -> p t c", p=128)
    xk_r = xk_d.rearrange("(t p) c -> p t c", p=128)
    xv_r = xv_d.rearrange("(t p) c -> p t c", p=128)
    wT_r = wT_d.rearrange("(c p) o -> p c o", p=128)
    woT_r = woT_d.rearrange("(c p) o -> p c o", p=128)
    y_r = y_d.rearrange("(t p) c -> p t c", p=128)

    with tile.TileContext(nc) as tc:
        with (
            tc.tile_pool(name="singles", bufs=1) as singles,
            tc.tile_pool(name="work", bufs=4) as work,
            tc.tile_pool(name="psum", bufs=4, space="PSUM") as psum,
            tc.tile_pool(name="psums", bufs=2, space="PSUM") as psums,
            tc.tile_pool(name="psumt", bufs=2, space="PSUM") as psumt,
        ):
            # ---- constants ----
            wT = singles.tile([128, IC, GO], fp16, tag="wT")
            nc.sync.dma_start(out=wT, in_=wT_r)
            woT = singles.tile([128, OC, DIM], fp16, tag="woT")
            nc.sync.dma_start(out=woT, in_=woT_r)
            e2 = singles.tile([128, OC, 128], fp16, tag="e2")
            nc.sync.dma_start(out=e2, in_=e2_d)
            ident = singles.tile([128, 128], fp16, tag="ident")
            nc.sync.dma_start(out=ident, in_=id_d)
            eps_t = singles.tile([128, 1], f32, tag="eps")
            nc.vector.memset(eps_t, LN_EPS)
            sblk = singles.tile([128, OC, 128], fp16, tag="sblk")
            nc.gpsimd.memset(sblk, 0.0)

            # persistent tensors
            zq = singles.tile([128, NT, DIM], fp16, tag="zq")
            zk = singles.tile([128, NT, DIM], fp16, tag="zk")
            zv = singles.tile([128, NT, DIM], fp16, tag="zv")
            zqT = singles.tile([128, IC, N], fp16, tag="zqT")
            zkT = singles.tile([128, IC, N], fp16, tag="zkT")
            fqT = singles.tile([128, OC, N], fp16, tag="fqT")
            kn = singles.tile([128, NT, GO], fp16, tag="kn")
            gt = singles.tile([128, IC, GO], fp16, tag="gt")
            rr = singles.tile([128, OC, N], fp16, tag="rr")
            ot = singles.tile([128, OC, N], fp16, tag="ot")
            rq = singles.tile([128, N], fp16, tag="rq")

            def load_lnorm(x_r, z, affine_on_scalar):
                """DMA-load one tensor (f32->fp16 cast) and layer-normalize
                in place into z."""
                rstd = work.tile([128, NT], f32, tag="rstd")
                nmr = work.tile([128, NT], f32, tag="nmr")
                for c4 in range(4):
                    nc.gpsimd.dma_start(
                        out=z[:, 4 * c4 : 4 * c4 + 4, :],
                        in_=x_r[:, 4 * c4 : 4 * c4 + 4, :],
                    )
                mvall = work.tile([128, NT, 2], f32, tag="mvall")
                for t in range(NT):
                    st = work.tile([128, 6], f32, tag="bnst")
                    nc.vector.bn_stats(out=st, in_=z[:, t, :])
                    nc.vector.bn_aggr(out=mvall[:, t, :], in_=st)
                # rstd = 1/sqrt(var+eps), nmr = -mu*rstd   (batched over tiles)
                sd = work.tile([128, NT], f32, tag="sd")
                nc.scalar.activation(
                    out=sd, in_=mvall[:, :, 1], func=AF.Sqrt, bias=eps_t[:, 0:1]
                )
                nc.vector.reciprocal(out=rstd, in_=sd)
                nc.vector.tensor_tensor(
                    out=nmr, in0=mvall[:, :, 0], in1=rstd, op=OP.mult
                )
                nc.vector.tensor_scalar_mul(out=nmr, in0=nmr, scalar1=-1.0)
                for t in range(NT):
                    if affine_on_scalar:
                        nc.scalar.activation(
                            out=z[:, t, :], in_=z[:, t, :], func=AF.Identity,
                            bias=nmr[:, t : t + 1], scale=rstd[:, t : t + 1],
                        )
                    else:
                        nc.vector.tensor_scalar(
                            out=z[:, t, :], in0=z[:, t, :],
                            scalar1=nmr[:, t : t + 1], scalar2=rstd[:, t : t + 1],
                            op0=OP.add, op1=OP.mult,
                        )

            def transpose_z(z, zT):
                """PE-transpose z [2048(n), 512(c)] -> zT [512(c), 2048(n)]."""
                for ic in range(IC):
                    for tg in range(4):
                        tp = psumt.tile([128, 512], fp16, tag="tp")
                        for j in range(4):
                            t = 4 * tg + j
                            nc.tensor.transpose(
                                tp[:, 128 * j : 128 * j + 128],
                                z[:, t, 128 * ic : 128 * ic + 128],
                                ident,
                            )
                        if (ic + tg) % 2 == 0:
                            nc.vector.tensor_copy(
                                out=zT[:, ic, 512 * tg : 512 * tg + 512], in_=tp
                            )
                        else:
                            nc.scalar.copy(
                                out=zT[:, ic, 512 * tg : 512 * tg + 512], in_=tp
                            )

            # ================= K side =================
            load_lnorm(xk_r, zk, affine_on_scalar=True)
            transpose_z(zk, zkT)

            # fk (n-major) + row norms + kn = fk * rk
            for mc in range(NT):
                fkp = psum.tile([128, GO], f32, tag="ps")
                for ic in range(IC):
                    nc.tensor.matmul(
                        fkp,
                        lhsT=zkT[:, ic, 128 * mc : 128 * mc + 128],
                        rhs=wT[:, ic, :],
                        start=(ic == 0),
                        stop=(ic == IC - 1),
                    )
                sqk = work.tile([128, GO], fp16, tag="sqk")
                nc.scalar.activation(out=sqk, in_=fkp, func=AF.Square)
                nsq = work.tile([128, GH], f32, tag="nsq")
                nc.vector.reduce_sum(
                    out=nsq,
                    in_=sqk.rearrange("p (h d) -> p h d", h=GH),
                    axis=AX.X,
                )
                nrm = work.tile([128, GH], f32, tag="nrm")
                nc.scalar.activation(out=nrm, in_=nsq, func=AF.Sqrt)
                rk = work.tile([128, GH], f32, tag="rk")
                nc.vector.reciprocal(out=rk, in_=nrm)
                for hh in range(GH):
                    nc.vector.tensor_scalar_mul(
                        out=kn[:, mc, 64 * hh : 64 * hh + 64],
                        in0=fkp[:, 64 * hh : 64 * hh + 64],
                        scalar1=rk[:, hh : hh + 1],
                    )

            # ================= Q side =================
            load_lnorm(xq_r, zq, affine_on_scalar=True)
            transpose_z(zq, zqT)

            # fq^T (feature-major) + q norms (indicator matmul)
            for ng in range(4):
                nqp = psum.tile([128, 512], f32, tag="ps")
                for oc in range(OC):
                    fqp = psum.tile([128, 512], f32, tag="ps")
                    for ic in range(IC):
                        nc.tensor.matmul(
                            fqp,
                            lhsT=wT[:, ic, 128 * oc : 128 * oc + 128],
                            rhs=zqT[:, ic, 512 * ng : 512 * ng + 512],
                            start=(ic == 0),
                            stop=(ic == IC - 1),
                        )
                    if oc % 2 == 0:
                        nc.vector.tensor_copy(
                            out=fqT[:, oc, 512 * ng : 512 * ng + 512], in_=fqp
                        )
                    else:
                        nc.scalar.copy(
                            out=fqT[:, oc, 512 * ng : 512 * ng + 512], in_=fqp
                        )
                    sqq = work.tile([128, 512], fp16, tag="sqq")
                    nc.vector.tensor_tensor(
                        out=sqq,
                        in0=fqT[:, oc, 512 * ng : 512 * ng + 512],
                        in1=fqT[:, oc, 512 * ng : 512 * ng + 512],
                        op=OP.mult,
                    )
                    nc.tensor.matmul(
                        nqp,
                        lhsT=e2[:, oc, :],
                        rhs=sqq,
                        start=(oc == 0),
                        stop=(oc == OC - 1),
                    )
                nqs = work.tile([128, 512], f32, tag="nqs")
                nc.scalar.activation(
                    out=nqs, in_=nqp, func=AF.Sqrt, bias=eps_t[:, 0:1]
                )
                with nc.allow_low_precision(reason="rq is a final scale; fp16 ok"):
                    nc.vector.reciprocal(
                        out=rq[:, 512 * ng : 512 * ng + 512], in_=nqs
                    )

            # R = rq broadcast over the 64 d-lanes of each head.
            # (bounce through DRAM: partition-step-0 reads are only legal
            # from DRAM sources)
            nc.sync.dma_start(out=rqd, in_=rq)
            for hh in range(GH):
                oc, loc = hh // 2, hh % 2
                row = rqd[32 * hh : 32 * hh + 1, :]
                src = bass.AP(
                    tensor=row.tensor,
                    offset=row.offset,
                    ap=[[0, 64]] + [list(d) for d in row.ap[1:]],
                )
                nc.gpsimd.dma_start(
                    out=rr[64 * loc : 64 * loc + 64, oc, :], in_=src
                )

            # ================= V side =================
            load_lnorm(xv_r, zv, affine_on_scalar=False)

            # G^T[in, (h,d1)] = sum_m z_v[m, in] * kn[m, (h,d1)]
            for ic in range(IC):
                gtp = psum.tile([128, GO], f32, tag="ps")
                for mc in range(NT):
                    nc.tensor.matmul(
                        gtp,
                        lhsT=zv[:, mc, 128 * ic : 128 * ic + 128],
                        rhs=kn[:, mc, :],
                        start=(mc == 0),
                        stop=(mc == NT - 1),
                    )
                nc.vector.tensor_copy(out=gt[:, ic, :], in_=gtp)

            # S_h = G_h @ W'_vh^T  ([64,64] each), laid into block-diagonal sblk
            for c in range(OC):
                sp = psums.tile([128, 64], f32, tag="sp")
                for loc in range(2):
                    hh = 2 * c + loc
                    for ic in range(IC):
                        nc.tensor.matmul(
                            sp[64 * loc : 64 * loc + 64, :],
                            lhsT=gt[:, ic, 64 * hh : 64 * hh + 64],
                            rhs=wT[:, ic, 64 * hh : 64 * hh + 64],
                            start=(ic == 0),
                            stop=(ic == IC - 1),
                        )
                nc.vector.tensor_copy(out=sblk[0:64, c, 0:64], in_=sp[0:64, :])
                nc.vector.tensor_copy(out=sblk[64:128, c, 64:128], in_=sp[64:128, :])

            # ================= U / O / Y =================
            for c in range(OC):
                for ng in range(4):
                    up = psum.tile([128, 512], f32, tag="ps")
                    nc.tensor.matmul(
                        up,
                        lhsT=sblk[:, c, :],
                        rhs=fqT[:, c, 512 * ng : 512 * ng + 512],
                        start=True,
                        stop=True,
                    )
                    nc.vector.tensor_tensor(
                        out=ot[:, c, 512 * ng : 512 * ng + 512],
                        in0=up,
                        in1=rr[:, c, 512 * ng : 512 * ng + 512],
                        op=OP.mult,
                    )

            if debug:
                for nm, t in [
                    ("dzk", zk), ("dzq", zq), ("dzv", zv), ("dzkT", zkT),
                    ("dzqT", zqT), ("dfqT", fqT), ("dkn", kn), ("dgt", gt),
                    ("dsblk", sblk), ("drq", rq), ("drr", rr), ("dot", ot),
                ]:
                    nc.sync.dma_start(out=dbg[nm], in_=t)

            for n4 in range(4):
                yb = work.tile([128, 4, DIM], fp16, tag="yb")
                for j in range(4):
                    ncn = 4 * n4 + j
                    yp = psum.tile([128, DIM], f32, tag="ps")
                    for oc in range(OC):
                        nc.tensor.matmul(
                            yp,
                            lhsT=ot[:, oc, 128 * ncn : 128 * ncn + 128],
                            rhs=woT[:, oc, :],
                            start=(oc == 0),
                            stop=(oc == OC - 1),
                        )
                    if j % 2 == 0:
                        nc.vector.tensor_copy(out=yb[:, j, :], in_=yp)
                    else:
                        nc.scalar.copy(out=yb[:, j, :], in_=yp)
                nc.sync.dma_start(out=y_r[:, 4 * n4 : 4 * n4 + 4, :], in_=yb)

    nc.compile()
    return nc


def _get_program():
    if "nc" not in _CACHE:
        _CACHE["nc"] = _build_program()
    return _CACHE["nc"]


def _host_inputs(q, k, v, ln_gamma, w_in, w_out):
    """Build the 8 per-core input maps."""
    W = (w_in * ln_gamma[None, :]).astype(np.float32)  # fold gamma
    e2 = np.zeros((128, OC, 128), dtype=np.float16)
    for p in range(128):
        for c in range(OC):
            e2[p, c, 32 * (2 * c + p // 64)] = 1.0
    ident = np.eye(128, dtype=np.float16)
    in_maps = []
    for core in range(8):
        b, g = core // 2, core % 2
        Wg = W[g * GO : (g + 1) * GO, :]          # [256, 512]
        wT = np.ascontiguousarray(Wg.T).astype(np.float16)      # [512, 256]
        woT = np.ascontiguousarray(w_out[:, g * GO : (g + 1) * GO].T).astype(
            np.float16
        )  # [256, 512]
        in_maps.append(
            {
                "xq": np.ascontiguousarray(q[b]).astype(np.float32),
                "xk": np.ascontiguousarray(k[b]).astype(np.float32),
                "xv": np.ascontiguousarray(v[b]).astype(np.float32),
                "wT": wT,
                "woT": woT,
                "e2": e2,
                "ident": ident,
            }
        )
    return in_maps


def _numpy_fallback(q, k, v, ln_gamma, ln_beta, w_in, w_out, b_out):
    """Exact reference in numpy (used only when ln_beta != 0)."""
    x = np.stack([q, k, v])  # [3, b, n, d]
    mu = x.mean(-1, keepdims=True)
    var = ((x - mu) ** 2).mean(-1, keepdims=True)
    xn = (x - mu) / np.sqrt(var + LN_EPS) * ln_gamma + ln_beta
    f = xn @ w_in.T  # [3, b, n, inner]
    f = f.reshape(3, B, N, HEADS, DIM_HEAD).transpose(0, 3, 1, 2, 4)
    fq, fk, fv = f[0], f[1], f[2]
    qn = fq / np.linalg.norm(fq, axis=-1, keepdims=True)
    knn = fk / np.linalg.norm(fk, axis=-1, keepdims=True)
    s = np.einsum("hbmd,hbme->hbde", knn, fv)
    out = np.einsum("hbnd,hbde->hbne", qn, s)
    out = out.transpose(1, 2, 0, 3).reshape(B, N, HEADS * DIM_HEAD)
    return (out @ w_out.T + b_out).astype(np.float32)


def kernel(q, k, v, ln_gamma, ln_beta, w_in, w_out, b_out):
    q = np.asarray(q, dtype=np.float32)
    k = np.asarray(k, dtype=np.float32)
    v = np.asarray(v, dtype=np.float32)
    ln_gamma = np.asarray(ln_gamma, dtype=np.float32)
    ln_beta = np.asarray(ln_beta, dtype=np.float32)
    w_in = np.asarray(w_in, dtype=np.float32)
    w_out = np.asarray(w_out, dtype=np.float32)
    b_out = np.asarray(b_out, dtype=np.float32)

    if np.any(ln_beta != 0.0):
        return _numpy_fallback(q, k, v, ln_gamma, ln_beta, w_in, w_out, b_out)

    from concourse.bass_utils import run_bass_kernel_spmd

    nc = _get_program()
    in_maps = _host_inputs(q, k, v, ln_gamma, w_in, w_out)
    trace = bool(int(os.environ.get("KERNEL_TRACE", "0")))
    res = run_bass_kernel_spmd(
        nc, in_maps, core_ids=list(range(8)), trace=trace
    )
    _CACHE["last_results"] = res

    out = np.empty((B, N, DIM), dtype=np.float32)
    for b in range(B):
        y0 = res.results[2 * b]["y"].astype(np.float32)
        y1 = res.results[2 * b + 1]["y"].astype(np.float32)
        out[b] = y0 + y1 + b_out[None, :]
    return out


# revision 13
# speedup vs baseline: 1.1003x; 1.1003x over previous
"""Trainium2 Bass kernel for nn_Attention_74766790689469.

Computation (per batch b):
    z   = LayerNorm(x) (gamma folded into W, beta==0 fast path)
    f_t = z_t @ W'^T               (t in {q,k,v}, W' = w_in * gamma)
    qn  = fq / |fq|_rows ; kn = fk / |fk|_rows    (per head, d=64)
    out_h = qn_h @ (kn_h^T @ fv_h)                (no softmax -> associativity)
    y   = concat_h(out_h) @ w_out^T + b_out

Sharding: 8 cores = 4 batches x 2 head-groups (4 heads each). Each core
computes a partial y for its (b, g); host sums the two group partials.

Device-side structure (per core), matmul operands fp16, PSUM f32:
  - interleaved chunked loads k/v/q (f32 -> fp16 SWDGE cast)
  - per-chunk LN: bn_stats/bn_aggr, rstd via Abs_reciprocal_sqrt,
    affine in place alternating DVE/ACT
  - PE-transpose z_q, z_k -> feature-major [512, 2048] (per-chunk)
  - fk = z_k^T-proj (n-major), row norms, kn = fk * rk
  - G^T = sum_m z_v[m,:]^T kn[m,:]  (contracts over m -> no z_v transpose)
  - S_h = G_h @ W'_vh^T [64,64]; U^T via blockdiag(S) matmul with fq^T
  - O^T = U^T * R (R = q-norm reciprocals broadcast via DRAM bounce)
  - y = O^T-proj with w_out^T, fp16 output
"""

import os
import numpy as np

HEADS = 8
DIM_HEAD = 64
DIM = 512
N = 2048
B = 4
LN_EPS = 1e-5
GH = 4            # heads per group
GO = GH * DIM_HEAD  # 256 outputs per group
NT = N // 128     # 16 row tiles
IC = DIM // 128   # 4 input-feature chunks
OC = GO // 128    # 2 output-feature chunks per group

_CACHE = {}


def _build_program(debug=False):
    """Build the single-core Bass program (same for every core)."""
    import concourse.bass as bass
    import concourse.bacc as bacc
    import concourse.tile as tile
    from concourse import mybir

    fp16 = mybir.dt.float16
    f32 = mybir.dt.float32
    AF = mybir.ActivationFunctionType
    OP = mybir.AluOpType
    AX = mybir.AxisListType

    nc = bacc.Bacc(
        "TRN2",
        target_bir_lowering=False,
        debug=False,
        enable_asserts=False,
        num_devices=8,
    )

    # ---- DRAM I/O ----
    xq_d = nc.dram_tensor("xq", [N, DIM], f32, kind="ExternalInput").ap()
    xk_d = nc.dram_tensor("xk", [N, DIM], f32, kind="ExternalInput").ap()
    xv_d = nc.dram_tensor("xv", [N, DIM], f32, kind="ExternalInput").ap()
    wT_d = nc.dram_tensor("wT", [DIM, GO], fp16, kind="ExternalInput").ap()
    woT_d = nc.dram_tensor("woT", [GO, DIM], fp16, kind="ExternalInput").ap()
    e2_d = nc.dram_tensor("e2", [128, OC, 128], fp16, kind="ExternalInput").ap()
    id_d = nc.dram_tensor("ident", [128, 128], fp16, kind="ExternalInput").ap()
    y_d = nc.dram_tensor("y", [N, DIM], fp16, kind="ExternalOutput").ap()
    rqd = nc.dram_tensor("rqd", [128, N], fp16).ap()  # internal scratch
    dbg = {}
    if debug:
        for nm, shp in [
            ("dzk", [128, NT, DIM]), ("dzq", [128, NT, DIM]),
            ("dzv", [128, NT, DIM]), ("dzkT", [128, IC, N]),
            ("dzqT", [128, IC, N]), ("dfqT", [128, OC, N]),
            ("dkn", [128, NT, GO]), ("dgt", [128, IC, GO]),
            ("dsblk", [128, OC, 128]), ("drq", [128, N]),
            ("drr", [128, OC, N]), ("dot", [128, OC, N]),
        ]:
            dbg[nm] = nc.dram_tensor(nm, shp, fp16, kind="ExternalOutput").ap()

    # tiled views  (row n = 128*t + p)
    xq_r = xq_d.rearrange("(t p) c -> p t c", p=128)
    xk_r = xk_d.rearrange("(t p) c -> p t c", p=128)
    xv_r = xv_d.rearrange("(t p) c -> p t c", p=128)
    wT_r = wT_d.rearrange("(c p) o -> p c o", p=128)
    woT_r = woT_d.rearrange("(c p) o -> p c o", p=128)
    y_r = y_d.rearrange("(t p) c -> p t c", p=128)

    with tile.TileContext(nc) as tc:
        with (
            tc.tile_pool(name="singles", bufs=1) as singles,
            tc.tile_pool(name="work", bufs=4) as work,
            tc.tile_pool(name="psum", bufs=4, space="PSUM") as psum,
            tc.tile_pool(name="psums", bufs=2, space="PSUM") as psums,
            tc.tile_pool(name="psumt", bufs=2, space="PSUM") as psumt,
        ):
            # ---- constants ----
            wT = singles.tile([128, IC, GO], fp16, tag="wT")
            nc.sync.dma_start(out=wT, in_=wT_r)
            woT = singles.tile([128, OC, DIM], fp16, tag="woT")
            nc.sync.dma_start(out=woT, in_=woT_r)
            e2 = singles.tile([128, OC, 128], fp16, tag="e2")
            nc.sync.dma_start(out=e2, in_=e2_d)
            ident = singles.tile([128, 128], fp16, tag="ident")
            nc.sync.dma_start(out=ident, in_=id_d)
            eps_t = singles.tile([128, 1], f32, tag="eps")
            nc.vector.memset(eps_t, LN_EPS)
            sblk = singles.tile([128, OC, 128], fp16, tag="sblk")
            nc.gpsimd.memset(sblk, 0.0)

            # persistent tensors
            zq = singles.tile([128, NT, DIM], fp16, tag="zq")
            zk = singles.tile([128, NT, DIM], fp16, tag="zk")
            zv = singles.tile([128, NT, DIM], fp16, tag="zv")
            zqT = singles.tile([128, IC, N], fp16, tag="zqT")
            zkT = singles.tile([128, IC, N], fp16, tag="zkT")
            fqT = singles.tile([128, OC, N], fp16, tag="fqT")
            kn = singles.tile([128, NT, GO], fp16, tag="kn")
            gt = singles.tile([128, IC, GO], fp16, tag="gt")
            rr = singles.tile([128, OC, N], fp16, tag="rr")
            ot = singles.tile([128, OC, N], fp16, tag="ot")
            rq = singles.tile([128, N], fp16, tag="rq")

            # interleaved chunk loads: k0 v0 q0 k1 v1 q1 ... so every
            # tensor's early chunks land early
            for c4 in range(4):
                for x_r, z in ((xk_r, zk), (xv_r, zv), (xq_r, zq)):
                    nc.gpsimd.dma_start(
                        out=z[:, 4 * c4 : 4 * c4 + 4, :],
                        in_=x_r[:, 4 * c4 : 4 * c4 + 4, :],
                    )

            def ln_chunk(z, c4, phase):
                """LayerNorm 4 row-tiles of z in place (chunk c4)."""
                mv = work.tile([128, 4, 2], f32, tag="mv")
                for j in range(4):
                    t = 4 * c4 + j
                    st = work.tile([128, 6], f32, tag="bnst")
                    nc.vector.bn_stats(out=st, in_=z[:, t, :])
                    nc.vector.bn_aggr(out=mv[:, j, :], in_=st)
                rstd4 = work.tile([128, 4], f32, tag="rstd4")
                nc.scalar.activation(
                    out=rstd4, in_=mv[:, :, 1],
                    func=AF.Abs_reciprocal_sqrt, bias=eps_t[:, 0:1],
                )
                nmr4 = work.tile([128, 4], f32, tag="nmr4")
                nc.vector.scalar_tensor_tensor(
                    out=nmr4, in0=mv[:, :, 0], scalar=-1.0, in1=rstd4,
                    op0=OP.mult, op1=OP.mult,
                )
                for j in range(4):
                    t = 4 * c4 + j
                    if (t + phase) % 2 == 0:
                        nc.scalar.activation(
                            out=z[:, t, :], in_=z[:, t, :], func=AF.Identity,
                            bias=nmr4[:, j : j + 1], scale=rstd4[:, j : j + 1],
                        )
                    else:
                        nc.vector.tensor_scalar(
                            out=z[:, t, :], in0=z[:, t, :],
                            scalar1=nmr4[:, j : j + 1],
                            scalar2=rstd4[:, j : j + 1],
                            op0=OP.add, op1=OP.mult,
                        )

            def tp_chunk(z, zT, tg):
                """PE-transpose row-tile group tg of z into zT columns."""
                for ic in range(IC):
                    tp = psumt.tile([128, 512], fp16, tag="tp")
                    for j in range(4):
                        t = 4 * tg + j
                        nc.tensor.transpose(
                            tp[:, 128 * j : 128 * j + 128],
                            z[:, t, 128 * ic : 128 * ic + 128],
                            ident,
                        )
                    if (ic + tg) % 2 == 0:
                        nc.vector.tensor_copy(
                            out=zT[:, ic, 512 * tg : 512 * tg + 512], in_=tp
                        )
                    else:
                        nc.scalar.copy(
                            out=zT[:, ic, 512 * tg : 512 * tg + 512], in_=tp
                        )

            for c4 in range(4):
                ln_chunk(zk, c4, 0)
                ln_chunk(zv, c4, 1)
                ln_chunk(zq, c4, 0)
                tp_chunk(zk, zkT, c4)
                tp_chunk(zq, zqT, c4)

            # ====== fk (n-major) + row norms + kn = fk * rk ======
            for mc in range(NT):
                fkp = psum.tile([128, GO], f32, tag="ps")
                for ic in range(IC):
                    nc.tensor.matmul(
                        fkp,
                        lhsT=zkT[:, ic, 128 * mc : 128 * mc + 128],
                        rhs=wT[:, ic, :],
                        start=(ic == 0),
                        stop=(ic == IC - 1),
                    )
                sqk = work.tile([128, GO], fp16, tag="sqk")
                nc.scalar.activation(out=sqk, in_=fkp, func=AF.Square)
                nsq = work.tile([128, GH], f32, tag="nsq")
                nc.vector.reduce_sum(
                    out=nsq,
                    in_=sqk.rearrange("p (h d) -> p h d", h=GH),
                    axis=AX.X,
                )
                rk4 = work.tile([128, GH], fp16, tag="rk4")
                nc.scalar.activation(
                    out=rk4, in_=nsq, func=AF.Abs_reciprocal_sqrt,
                    bias=eps_t[:, 0:1],
                )
                import concourse.bass as _b
                rk_b = _b.AP(
                    tensor=rk4.tensor,
                    offset=rk4.offset,
                    ap=[list(rk4.ap[0]), list(rk4.ap[1]), [0, 64]],
                )
                nc.vector.tensor_tensor(
                    out=kn[:, mc, :].rearrange("p (h d) -> p h d", h=GH),
                    in0=fkp.rearrange("p (h d) -> p h d", h=GH),
                    in1=rk_b,
                    op=OP.mult,
                )

            # ====== fq^T (feature-major) + q norms (indicator matmul) ======
            for ng in range(4):
                nqp = psum.tile([128, 512], f32, tag="ps")
                for oc in range(OC):
                    fqp = psum.tile([128, 512], f32, tag="ps")
                    for ic in range(IC):
                        nc.tensor.matmul(
                            fqp,
                            lhsT=wT[:, ic, 128 * oc : 128 * oc + 128],
                            rhs=zqT[:, ic, 512 * ng : 512 * ng + 512],
                            start=(ic == 0),
                            stop=(ic == IC - 1),
                        )
                    if oc % 2 == 0:
                        nc.vector.tensor_copy(
                            out=fqT[:, oc, 512 * ng : 512 * ng + 512], in_=fqp
                        )
                    else:
                        nc.scalar.copy(
                            out=fqT[:, oc, 512 * ng : 512 * ng + 512], in_=fqp
                        )
                    sqq = work.tile([128, 512], fp16, tag="sqq")
                    nc.vector.tensor_tensor(
                        out=sqq,
                        in0=fqT[:, oc, 512 * ng : 512 * ng + 512],
                        in1=fqT[:, oc, 512 * ng : 512 * ng + 512],
                        op=OP.mult,
                    )
                    nc.tensor.matmul(
                        nqp,
                        lhsT=e2[:, oc, :],
                        rhs=sqq,
                        start=(oc == 0),
                        stop=(oc == OC - 1),
                    )
                # rq = 1/sqrt(nq + eps) directly on ACT
                nc.scalar.activation(
                    out=rq[:, 512 * ng : 512 * ng + 512], in_=nqp,
                    func=AF.Abs_reciprocal_sqrt, bias=eps_t[:, 0:1],
                )

            # R = rq broadcast over the 64 d-lanes of each head
            # (bounce through DRAM: partition-step-0 reads need DRAM source)
            nc.sync.dma_start(out=rqd, in_=rq)
            import concourse.bass as _bass
            for hh in range(GH):
                oc, loc = hh // 2, hh % 2
                row = rqd[32 * hh : 32 * hh + 1, :]
                src = _bass.AP(
                    tensor=row.tensor,
                    offset=row.offset,
                    ap=[[0, 64]] + [list(d) for d in row.ap[1:]],
                )
                nc.gpsimd.dma_start(
                    out=rr[64 * loc : 64 * loc + 64, oc, :], in_=src
                )

            # ====== G^T[in, (h,d1)] = sum_m z_v[m, in] * kn[m, (h,d1)] ======
            for ic in range(IC):
                gtp = psum.tile([128, GO], f32, tag="ps")
                for mc in range(NT):
                    nc.tensor.matmul(
                        gtp,
                        lhsT=zv[:, mc, 128 * ic : 128 * ic + 128],
                        rhs=kn[:, mc, :],
                        start=(mc == 0),
                        stop=(mc == NT - 1),
                    )
                nc.vector.tensor_copy(out=gt[:, ic, :], in_=gtp)

            # ====== S_h = G_h @ W'_vh^T, laid into block-diagonal sblk ======
            for c in range(OC):
                sp = psums.tile([128, 64], f32, tag="sp")
                for loc in range(2):
                    hh = 2 * c + loc
                    for ic in range(IC):
                        nc.tensor.matmul(
                            sp[64 * loc : 64 * loc + 64, :],
                            lhsT=gt[:, ic, 64 * hh : 64 * hh + 64],
                            rhs=wT[:, ic, 64 * hh : 64 * hh + 64],
                            start=(ic == 0),
                            stop=(ic == IC - 1),
                        )
                nc.vector.tensor_copy(out=sblk[0:64, c, 0:64], in_=sp[0:64, :])
                nc.vector.tensor_copy(out=sblk[64:128, c, 64:128], in_=sp[64:128, :])

            # ====== U / O ======
            for c in range(OC):
                for ng in range(4):
                    up = psum.tile([128, 512], f32, tag="ps")
                    nc.tensor.matmul(
                        up,
                        lhsT=sblk[:, c, :],
                        rhs=fqT[:, c, 512 * ng : 512 * ng + 512],
                        start=True,
                        stop=True,
                    )
                    nc.vector.tensor_tensor(
                        out=ot[:, c, 512 * ng : 512 * ng + 512],
                        in0=up,
                        in1=rr[:, c, 512 * ng : 512 * ng + 512],
                        op=OP.mult,
                    )

            if debug:
                for nm, t in [
                    ("dzk", zk), ("dzq", zq), ("dzv", zv), ("dzkT", zkT),
                    ("dzqT", zqT), ("dfqT", fqT), ("dkn", kn), ("dgt", gt),
                    ("dsblk", sblk), ("drq", rq), ("drr", rr), ("dot", ot),
                ]:
                    nc.sync.dma_start(out=dbg[nm], in_=t)

            # ====== Y = O^T-proj ======
            for n4 in range(4):
                yb = work.tile([128, 4, DIM], fp16, tag="yb")
                for j in range(4):
                    ncn = 4 * n4 + j
                    yp = psum.tile([128, DIM], f32, tag="ps")
                    for oc in range(OC):
                        nc.tensor.matmul(
                            yp,
                            lhsT=ot[:, oc, 128 * ncn : 128 * ncn + 128],
                            rhs=woT[:, oc, :],
                            start=(oc == 0),
                            stop=(oc == OC - 1),
                        )
                    if j % 2 == 0:
                        nc.vector.tensor_copy(out=yb[:, j, :], in_=yp)
                    else:
                        nc.scalar.copy(out=yb[:, j, :], in_=yp)
                nc.sync.dma_start(out=y_r[:, 4 * n4 : 4 * n4 + 4, :], in_=yb)

    nc.compile()
    return nc


def _get_program():
    if "nc" not in _CACHE:
        _CACHE["nc"] = _build_program()
    return _CACHE["nc"]


def _host_inputs(q, k, v, ln_gamma, w_in, w_out):
    """Build the 8 per-core input maps."""
    W = (w_in * ln_gamma[None, :]).astype(np.float32)  # fold gamma
    e2 = np.zeros((128, OC, 128), dtype=np.float16)
    for p in range(128):
        for c in range(OC):
            e2[p, c, 32 * (2 * c + p // 64)] = 1.0
    ident = np.eye(128, dtype=np.float16)
    in_maps = []
    for core in range(8):
        b, g = core // 2, core % 2
        Wg = W[g * GO : (g + 1) * GO, :]          # [256, 512]
        wT = np.ascontiguousarray(Wg.T).astype(np.float16)      # [512, 256]
        woT = np.ascontiguousarray(w_out[:, g * GO : (g + 1) * GO].T).astype(
            np.float16
        )  # [256, 512]
        in_maps.append(
            {
                "xq": np.ascontiguousarray(q[b]).astype(np.float32),
                "xk": np.ascontiguousarray(k[b]).astype(np.float32),
                "xv": np.ascontiguousarray(v[b]).astype(np.float32),
                "wT": wT,
                "woT": woT,
                "e2": e2,
                "ident": ident,
            }
        )
    return in_maps


def _numpy_fallback(q, k, v, ln_gamma, ln_beta, w_in, w_out, b_out):
    """Exact reference in numpy (used only when ln_beta != 0)."""
    x = np.stack([q, k, v])  # [3, b, n, d]
    mu = x.mean(-1, keepdims=True)
    var = ((x - mu) ** 2).mean(-1, keepdims=True)
    xn = (x - mu) / np.sqrt(var + LN_EPS) * ln_gamma + ln_beta
    f = xn @ w_in.T  # [3, b, n, inner]
    f = f.reshape(3, B, N, HEADS, DIM_HEAD).transpose(0, 3, 1, 2, 4)
    fq, fk, fv = f[0], f[1], f[2]
    qn = fq / np.linalg.norm(fq, axis=-1, keepdims=True)
    knn = fk / np.linalg.norm(fk, axis=-1, keepdims=True)
    s = np.einsum("hbmd,hbme->hbde", knn, fv)
    out = np.einsum("hbnd,hbde->hbne", qn, s)
    out = out.transpose(1, 2, 0, 3).reshape(B, N, HEADS * DIM_HEAD)
    return (out @ w_out.T + b_out).astype(np.float32)


def kernel(q, k, v, ln_gamma, ln_beta, w_in, w_out, b_out):
    q = np.asarray(q, dtype=np.float32)
    k = np.asarray(k, dtype=np.float32)
    v = np.asarray(v, dtype=np.float32)
    ln_gamma = np.asarray(ln_gamma, dtype=np.float32)
    ln_beta = np.asarray(ln_beta, dtype=np.float32)
    w_in = np.asarray(w_in, dtype=np.float32)
    w_out = np.asarray(w_out, dtype=np.float32)
    b_out = np.asarray(b_out, dtype=np.float32)

    if np.any(ln_beta != 0.0):
        return _numpy_fallback(q, k, v, ln_gamma, ln_beta, w_in, w_out, b_out)

    from concourse.bass_utils import run_bass_kernel_spmd

    nc = _get_program()
    in_maps = _host_inputs(q, k, v, ln_gamma, w_in, w_out)
    trace = bool(int(os.environ.get("KERNEL_TRACE", "0")))
    res = run_bass_kernel_spmd(
        nc, in_maps, core_ids=list(range(8)), trace=trace
    )
    _CACHE["last_results"] = res

    out = np.empty((B, N, DIM), dtype=np.float32)
    for b in range(B):
        y0 = res.results[2 * b]["y"].astype(np.float32)
        y1 = res.results[2 * b + 1]["y"].astype(np.float32)
        out[b] = y0 + y1 + b_out[None, :]
    return out
